# revision 1
# baseline (speedup 1.0000x reference)
"""Causal self-attention Trainium2 kernel (B=4, T=4096, C=384, H=6).

Sharding: 8 cores = 4 batches x 2 head-groups (3 heads each). Each core
computes y_partial = attn(x[b], heads hg) @ w_proj[rows of hg]; the host
sums the two partials per batch (the "all-reduce after c_proj" done on
host during unshard).
"""

import numpy as np
from contextlib import ExitStack

import concourse.bass as bass
import concourse.tile as tile
from concourse import mybir
from concourse.bass_utils import run_bass_kernel_spmd
from concourse.masks import make_identity
from concourse.vector_clock import ScopedClock

F32 = mybir.dt.float32
BF16 = mybir.dt.bfloat16
EXP = mybir.ActivationFunctionType.Exp
MULT = mybir.AluOpType.mult

B, T, C, H, D = 4, 4096, 384, 6, 64
HPC = 3            # heads per core
QT = 512           # q tile
KC = 128           # key chunk
SCALE = 1.0 / 8.0  # 1/sqrt(64)


# ---------------------------------------------------------------------------
# Workaround: neuronxcc CoreV3 rejects >2 sem waits on the Tile tail drain.
# Split the drain's waits into individual sync-engine wait instructions.
def _drain_and_barrier_split(self, tick_clock, wait_clock):
    nc = self.nc
    drain_inst = nc.sync.drain()
    wait_clock.add_sem_waits(
        drain_inst.ins, ScopedClock({None: tick_clock.global_clock})
    )
    si = drain_inst.ins.sync_info
    if si is not None and si.on_wait and len(si.on_wait) > 1:
        waits = list(si.on_wait)
        si.on_wait = []
        allocated = {h.name: h for h in self.sems.allocated().values()}
        for w in waits:
            h = allocated.get(w.ant_name)
            assert h is not None, f"no sem handle for drain wait {w.ant_name}"
            assert w.wait_mode == "sem-ge-imm", w.wait_mode
            nc.sync.wait_ge(h, w.wait_value)
    nc.all_engine_barrier()
    assert self.sems is not None
    popped = nc._tile_sem_poison_stack.pop()
    assert popped is self._sem_poison
    nc.clear_and_free_semaphores(list(self.sems.allocated().values()))
    nc.all_engine_barrier()


tile.TileContext._drain_and_barrier = _drain_and_barrier_split


MAX_WAITS = 1  # CoreV3 per-instruction sem-wait capacity (S3_LW holds only 1)


def _split_excess_waits(nc):
    """Hoist sem waits beyond MAX_WAITS onto same-engine NOPs inserted
    directly before the over-limit instruction (waits are order-free)."""
    for fn in nc.m.functions:
        for bb in fn.blocks:
            insts = list(bb.instructions)
            out = []
            changed = False
            for inst in insts:
                si = inst.sync_info
                if si is not None and si.on_wait and len(si.on_wait) > MAX_WAITS:
                    waits = list(si.on_wait)
                    excess, keep = waits[:-MAX_WAITS], waits[-MAX_WAITS:]
                    si.on_wait = keep
                    inst.sync_info = si
                    for i in range(0, len(excess), MAX_WAITS):
                        nop = mybir.InstNoOp(
                            name=f"{inst.name}-waitsplit-{i}", ins=[], outs=[]
                        )
                        nop.engine = inst.engine
                        nop.sync_info = mybir.SyncInfo(
                            on_wait=excess[i:i + MAX_WAITS], on_update=[]
                        )
                        nc.register_instruction(nop)
                        out.append(nop)
                    changed = True
                out.append(inst)
            if changed:
                bb.instructions = out
# ---------------------------------------------------------------------------


def build(t=T):
    nqt = t // QT          # q tiles
    nkc = t // KC          # key chunks
    ntb = t // QT          # token blocks for phase A (512 tokens each)

    nc = bass.Bass()
    x_d = nc.dram_tensor("xT16", [C, t], BF16, kind="ExternalInput")
    wq01_d = nc.dram_tensor("w_q01", [3, 128, 128], BF16, kind="ExternalInput")
    wk01_d = nc.dram_tensor("w_k01", [3, 128, 128], BF16, kind="ExternalInput")
    wq2_d = nc.dram_tensor("w_q2", [3, 128, 64], BF16, kind="ExternalInput")
    wk2_d = nc.dram_tensor("w_k2", [3, 128, 64], BF16, kind="ExternalInput")
    wv_d = nc.dram_tensor("w_v", [3, 128, 192], BF16, kind="ExternalInput")
    wo_d = nc.dram_tensor("w_o", [3, 64, 384], BF16, kind="ExternalInput")
    mask_d = nc.dram_tensor("masks", [4, 128, QT], BF16, kind="ExternalInput")
    y_d = nc.dram_tensor("y", [t, C], F32, kind="ExternalOutput")
    # scratch for transposing the softmax denominator row into columns
    l_d = nc.dram_tensor("lscratch", [t // QT, 3, QT], F32)

    with tile.TileContext(nc) as tc, ExitStack() as ctx:
        persist = ctx.enter_context(tc.tile_pool(name="persist", bufs=1))

        # weights / masks / identity
        wq01 = persist.tile([128, 3, 128], BF16)
        wk01 = persist.tile([128, 3, 128], BF16)
        wq2 = persist.tile([128, 3, 64], BF16)
        wk2 = persist.tile([128, 3, 64], BF16)
        wv = persist.tile([128, 3, 192], BF16)
        wo = persist.tile([64, 3, 384], BF16)
        for c in range(3):
            nc.sync.dma_start(out=wq01[:, c, :], in_=wq01_d[c])
            nc.sync.dma_start(out=wk01[:, c, :], in_=wk01_d[c])
            nc.sync.dma_start(out=wq2[:, c, :], in_=wq2_d[c])
            nc.sync.dma_start(out=wk2[:, c, :], in_=wk2_d[c])
            nc.sync.dma_start(out=wv[:, c, :], in_=wv_d[c])
            nc.sync.dma_start(out=wo[:, c, :], in_=wo_d[c])
        masks = persist.tile([128, 4, QT], BF16)
        for j in range(4):
            nc.sync.dma_start(out=masks[:, j, :], in_=mask_d[j])

        # persistent activations (bf16)
        qT01 = persist.tile([128, t], BF16)   # rows 0:64 h0 qT, 64:128 h1 qT
        kT01 = persist.tile([128, t], BF16)
        # head 2 q/k duplicated into both partition halves so chunk pairs
        # can run as concurrent row-group-packed matmuls
        qT2 = persist.tile([128, t], BF16)
        kT2 = persist.tile([128, t], BF16)
        vsb = persist.tile([128, nkc, 3, 65], BF16)  # [keys, chunk, head, d|one]
        nc.vector.memset(vsb[:, :, :, 64:65], 1.0)
        with (
            tc.tile_pool(name="xt", bufs=3) as xt_p,
            tc.tile_pool(name="ps", bufs=3, space="PSUM") as ps_p,
            tc.tile_pool(name="ps_att", bufs=1, space="PSUM") as ps_att,
            tc.tile_pool(name="ps_y", bufs=1, space="PSUM") as ps_y,
            tc.tile_pool(name="pth01", bufs=1) as pth01_p,
            tc.tile_pool(name="pth2", bufs=2) as pth2_p,
            tc.tile_pool(name="attn", bufs=6) as attn_p,
            tc.tile_pool(name="lrow", bufs=3) as lrow_p,
            tc.tile_pool(name="lcol", bufs=6) as lcol_p,
            tc.tile_pool(name="yout", bufs=3) as yout_p,
        ):
            def emit_cproj(prev):
                # c_proj of the PREVIOUS q tile, emitted into this tile's
                # stream so its PE/DVE tail overlaps the exp pipeline
                pqt, p_attn, p_linv = prev
                pq0 = pqt * QT
                for s in range(4):
                    ysb = yout_p.tile([128, C], F32, tag="ysb", name="ysb")
                    for h in range(3):
                        yp = ps_y.tile([128, C], F32, tag="y", name="yp")
                        nc.tensor.matmul(
                            yp[:],
                            p_attn[h][:, s * 128:(s + 1) * 128],
                            wo[:, h, :],
                            start=True, stop=True,
                        )
                        sc = p_linv[h][:, s:s + 1]
                        if h == 0:
                            nc.vector.tensor_scalar(
                                out=ysb[:], in0=yp[:], scalar1=sc,
                                scalar2=None, op0=MULT,
                            )
                        else:
                            nc.vector.scalar_tensor_tensor(
                                out=ysb[:], in0=yp[:], scalar=sc, in1=ysb[:],
                                op0=MULT, op1=mybir.AluOpType.add,
                            )
                    nc.sync.dma_start(
                        out=y_d[pq0 + s * 128:pq0 + (s + 1) * 128, :],
                        in_=ysb[:],
                    )

            for tb in range(ntb):
                # ---------- phase A block tb: q/k/v projections ----------
                xT = xt_p.tile([128, 3, QT], BF16, tag="xt", name="xT")
                for c in range(3):
                    nc.sync.dma_start(
                        out=xT[:, c, :],
                        in_=x_d[c * 128:(c + 1) * 128, tb * QT:(tb + 1) * QT],
                    )
                for w_sb, m, dst in (
                    (wq01, 128, qT01),
                    (wk01, 128, kT01),
                    (wq2, 64, qT2),
                    (wk2, 64, kT2),
                ):
                    ps = ps_p.tile([128, QT], F32, tag="ps", name="psqk")
                    for c in range(3):
                        nc.tensor.matmul(
                            ps[0:m, :], w_sb[:, c, 0:m], xT[:, c, :],
                            start=(c == 0), stop=(c == 2),
                        )
                    nc.vector.tensor_copy(
                        dst[0:m, tb * QT:(tb + 1) * QT], ps[0:m, :]
                    )
                nc.gpsimd.dma_start(
                    out=qT2[64:128, tb * QT:(tb + 1) * QT],
                    in_=qT2[0:64, tb * QT:(tb + 1) * QT],
                )
                nc.gpsimd.dma_start(
                    out=kT2[64:128, tb * QT:(tb + 1) * QT],
                    in_=kT2[0:64, tb * QT:(tb + 1) * QT],
                )
                for s in range(4):
                    psv = ps_p.tile([128, 3, 64], F32, tag="ps", name="psv")
                    for c in range(3):
                        nc.tensor.matmul(
                            psv[:, :, :].rearrange("p h d -> p (h d)"),
                            xT[:, c, s * 128:(s + 1) * 128],
                            wv[:, c, :],
                            start=(c == 0), stop=(c == 2),
                        )
                    nc.vector.tensor_copy(
                        vsb[:, tb * 4 + s, :, 0:64], psv[:, :, :]
                    )

                # ---------- phase B q-tile tb ----------
                qt = tb
                nch = 4 * (qt + 1)
                q0, q1 = qt * QT, (qt + 1) * QT

                # pth01: [chunk, head(h0|h1), q]; pth2: [chunk-pair, j, q]
                pth01 = pth01_p.tile([128, nkc, 2, QT], BF16, tag="pth01", name="pth01")
                pth2 = pth2_p.tile([128, nkc // 2, 2, QT], BF16, tag="pth2", name="pth2")

                # S^T + exp. h0/h1 write halves of ONE slot so both packed
                # matmuls share the same WAR wait and dispatch back-to-back
                # (concurrent row groups 0-1 / 2-3).
                for ck in range(nch):
                    ssx = ps_p.tile([128, 2, QT], F32, tag="ps", name="ssx")
                    nc.tensor.matmul(
                        ssx[:, 0, :],
                        kT01[0:64, ck * KC:(ck + 1) * KC],
                        qT01[0:64, q0:q1],
                        start=True, stop=True, tile_position=(0, 0),
                    )
                    nc.tensor.matmul(
                        ssx[:, 1, :],
                        kT01[64:128, ck * KC:(ck + 1) * KC],
                        qT01[64:128, q0:q1],
                        start=True, stop=True, tile_position=(64, 0),
                    )
                    nc.scalar.activation(
                        out=pth01[:, ck, :, :].rearrange("p j q -> p (j q)"),
                        in_=ssx[:, :, :].rearrange("p j q -> p (j q)"),
                        func=EXP, scale=SCALE,
                    )
                for g in range(nch // 2):
                    ssc = ps_p.tile([128, 2, QT], F32, tag="ps", name="ssc")
                    nc.tensor.matmul(
                        ssc[:, 0, :],
                        kT2[0:64, (2 * g) * KC:(2 * g + 1) * KC],
                        qT2[0:64, q0:q1],
                        start=True, stop=True, tile_position=(0, 0),
                    )
                    nc.tensor.matmul(
                        ssc[:, 1, :],
                        kT2[64:128, (2 * g + 1) * KC:(2 * g + 2) * KC],
                        qT2[64:128, q0:q1],
                        start=True, stop=True, tile_position=(64, 0),
                    )
                    nc.scalar.activation(
                        out=pth2[:, g, :, :].rearrange("p j q -> p (j q)"),
                        in_=ssc[:, :, :].rearrange("p j q -> p (j q)"),
                        func=EXP, scale=SCALE,
                    )

                # causal masks on the 4 diagonal chunks
                for j in range(4):
                    ck = 4 * qt + j
                    for sl in (
                        pth01[:, ck, 0, :], pth01[:, ck, 1, :],
                        pth2[:, ck // 2, ck % 2, :],
                    ):
                        nc.vector.tensor_tensor(
                            out=sl, in0=sl, in1=masks[:, j, :], op=MULT,
                        )

                # att^T accumulation + per-head normalization prep
                attn_tiles = []
                linv_tiles = []
                for h in range(3):
                    att = ps_att.tile([65, QT], F32, tag="att", name="att")
                    for ck in range(nch):
                        if h < 2:
                            rhs = pth01[:, ck, h, :]
                        else:
                            rhs = pth2[:, ck // 2, ck % 2, :]
                        nc.tensor.matmul(
                            att[:], vsb[:, ck, h, :], rhs,
                            start=(ck == 0), stop=(ck == nch - 1),
                        )
                    at = attn_p.tile([64, QT], BF16, tag="attn", name="at")
                    attn_tiles.append(at)
                    nc.vector.tensor_copy(at[:], att[0:64, :])
                    lrow = lrow_p.tile([65, QT], F32, tag="lrow", name="lrow")
                    nc.vector.tensor_copy(lrow[64:65, :], att[64:65, :])
                    nc.sync.dma_start(out=l_d[qt, h], in_=lrow[64:65, :])
                    lcol = lcol_p.tile([128, 4], F32, tag="lcol", name="lcol")
                    nc.sync.dma_start(
                        out=lcol[:],
                        in_=l_d[qt, h].rearrange("(s p) -> p s", p=128),
                    )
                    linv = lcol_p.tile([128, 4], F32, tag="linv", name="linv")
                    linv_tiles.append(linv)
                    nc.vector.reciprocal(linv[:], lcol[:])

                emit_cproj((qt, attn_tiles, linv_tiles))

    _split_excess_waits(nc)
    nc.finalize()
    return nc


_NC_CACHE = {}


def _get_nc(t=T):
    if t not in _NC_CACHE:
        _NC_CACHE[t] = build(t)
    return _NC_CACHE[t]


def _prep_core_inputs(x_b, w_attn, w_proj, hg, bf16):
    """Host-side shard prep for one core: batch x_b, head group hg (0/1)."""
    h0 = 3 * hg
    q = w_attn[:, 0:C]
    k = w_attn[:, C:2 * C]
    v = w_attn[:, 2 * C:3 * C]
    qcols = lambda h: q[:, h * D:(h + 1) * D]
    kcols = lambda h: k[:, h * D:(h + 1) * D]
    w_q01 = np.concatenate([qcols(h0), qcols(h0 + 1)], axis=1)      # [384,128]
    w_k01 = np.concatenate([kcols(h0), kcols(h0 + 1)], axis=1)
    w_q2 = qcols(h0 + 2)                                            # [384,64]
    w_k2 = kcols(h0 + 2)
    w_v = v[:, h0 * D:(h0 + 3) * D]                                 # [384,192]
    w_o = w_proj[h0 * D:(h0 + 3) * D, :]                            # [192,384]
    return {
        "xT16": np.ascontiguousarray(x_b.T, dtype=bf16),
        "w_q01": np.ascontiguousarray(w_q01.reshape(3, 128, 128), dtype=bf16),
        "w_k01": np.ascontiguousarray(w_k01.reshape(3, 128, 128), dtype=bf16),
        "w_q2": np.ascontiguousarray(w_q2.reshape(3, 128, 64), dtype=bf16),
        "w_k2": np.ascontiguousarray(w_k2.reshape(3, 128, 64), dtype=bf16),
        "w_v": np.ascontiguousarray(w_v.reshape(3, 128, 192), dtype=bf16),
        "w_o": np.ascontiguousarray(w_o.reshape(3, 64, 384), dtype=bf16),
    }


def _make_masks(bf16):
    m = np.zeros((4, 128, QT), dtype=np.float32)
    f = np.arange(QT)[None, :]
    p = np.arange(128)[:, None]
    for j in range(4):
        m[j] = (f - 128 * j >= p).astype(np.float32)
    return m.astype(bf16)


def kernel(x, w_attn, w_proj):
    import ml_dtypes
    bf16 = ml_dtypes.bfloat16

    x = np.asarray(x, dtype=np.float32)
    w_attn = np.asarray(w_attn, dtype=np.float32)
    w_proj = np.asarray(w_proj, dtype=np.float32)
    b, t, c = x.shape

    nc = _get_nc(t)
    masks = _make_masks(bf16)
    in_maps = []
    for core in range(8):
        im = _prep_core_inputs(x[core // 2], w_attn, w_proj, core % 2, bf16)
        im["masks"] = masks
        in_maps.append(im)

    res = run_bass_kernel_spmd(nc, in_maps, list(range(8)))
    out = np.empty((b, t, c), dtype=np.float32)
    for bb in range(b):
        out[bb] = res.results[2 * bb]["y"] + res.results[2 * bb + 1]["y"]
    return out



# revision 5
# speedup vs baseline: 1.1034x; 1.1034x over previous
"""Causal self-attention Trainium2 kernel (B=4, T=4096, C=384, H=6).

Sharding: 8 cores = 4 batches x 2 head-groups (3 heads each). Each core
computes y_partial = attn(x[b], heads hg) @ w_proj[rows of hg]; the host
sums the two partials per batch (the "all-reduce after c_proj" done on
host during unshard).

v2 pipeline (vs baseline): per-chunk interleaved AV with 3 persistent
PSUM attention accumulators, causal masking folded into the tensor
engine (identity x (-240*upper_tri) accumulate-matmuls in PSUM before
the exp), diagonal-chunk extent trimming, proj/c_proj interleaved as
fill work through a shared 1-bank PSUM slot, y-stores on the gpsimd DMA
queue, and an optional DVE bit-trick exp offload to unload the Scalar
engine (the global bottleneck).
"""

import numpy as np
from contextlib import ExitStack

import concourse.bass as bass
import concourse.tile as tile
from concourse import mybir
from concourse.bass_utils import run_bass_kernel_spmd
from concourse.vector_clock import ScopedClock

F32 = mybir.dt.float32
BF16 = mybir.dt.bfloat16
I16 = mybir.dt.int16
EXP = mybir.ActivationFunctionType.Exp
MULT = mybir.AluOpType.mult
ADD = mybir.AluOpType.add

B, T, C, H, D = 4, 4096, 384, 6, 64
HPC = 3            # heads per core
QT = 512           # q tile
KC = 128           # key chunk
SCALE = 1.0 / 8.0  # 1/sqrt(64)
NEGM = -240.0      # pre-scale mask bias: exp((S-240)/8) ~ 0

# DVE bit-trick exp (Schraudolph, direct to bf16 bits):
#   bf16_bits(exp(x/8)) ~= int16(x * (128*log2(e)/8) + K2)
SCH_K1 = 128.0 * 1.4426950408889634 / 8.0
SCH_K2 = 16250.75  # calibrated: ~3.3% max rel err on exp values
# Offload every SCH_DEN-th h2 pair-slot exp to the DVE (0 = off)
SCH_NUM = 0
SCH_DEN = 1


# ---------------------------------------------------------------------------
# Workaround: neuronxcc CoreV3 rejects >2 sem waits on the Tile tail drain.
# Split the drain's waits into individual sync-engine wait instructions.
def _drain_and_barrier_split(self, tick_clock, wait_clock):
    nc = self.nc
    drain_inst = nc.sync.drain()
    wait_clock.add_sem_waits(
        drain_inst.ins, ScopedClock({None: tick_clock.global_clock})
    )
    si = drain_inst.ins.sync_info
    if si is not None and si.on_wait and len(si.on_wait) > 1:
        waits = list(si.on_wait)
        si.on_wait = []
        allocated = {h.name: h for h in self.sems.allocated().values()}
        for w in waits:
            h = allocated.get(w.ant_name)
            assert h is not None, f"no sem handle for drain wait {w.ant_name}"
            assert w.wait_mode == "sem-ge-imm", w.wait_mode
            nc.sync.wait_ge(h, w.wait_value)
    nc.all_engine_barrier()
    assert self.sems is not None
    popped = nc._tile_sem_poison_stack.pop()
    assert popped is self._sem_poison
    nc.clear_and_free_semaphores(list(self.sems.allocated().values()))
    nc.all_engine_barrier()


tile.TileContext._drain_and_barrier = _drain_and_barrier_split


MAX_WAITS = 1  # CoreV3 per-instruction sem-wait capacity (S3_LW holds only 1)


def _split_excess_waits(nc):
    """Hoist sem waits beyond MAX_WAITS onto same-engine NOPs inserted
    directly before the over-limit instruction (waits are order-free)."""
    for fn in nc.m.functions:
        for bb in fn.blocks:
            insts = list(bb.instructions)
            out = []
            changed = False
            for inst in insts:
                si = inst.sync_info
                if si is not None and si.on_wait and len(si.on_wait) > MAX_WAITS:
                    waits = list(si.on_wait)
                    excess, keep = waits[:-MAX_WAITS], waits[-MAX_WAITS:]
                    si.on_wait = keep
                    inst.sync_info = si
                    for i in range(0, len(excess), MAX_WAITS):
                        nop = mybir.InstNoOp(
                            name=f"{inst.name}-waitsplit-{i}", ins=[], outs=[]
                        )
                        nop.engine = inst.engine
                        nop.sync_info = mybir.SyncInfo(
                            on_wait=excess[i:i + MAX_WAITS], on_update=[]
                        )
                        nc.register_instruction(nop)
                        out.append(nop)
                    changed = True
                out.append(inst)
            if changed:
                bb.instructions = out
# ---------------------------------------------------------------------------


def build(t=T):
    nqt = t // QT          # q tiles
    nkc = t // KC          # key chunks

    nc = bass.Bass()
    x_d = nc.dram_tensor("xT16", [C, t], BF16, kind="ExternalInput")
    wq01_d = nc.dram_tensor("w_q01", [3, 128, 128], BF16, kind="ExternalInput")
    wk01_d = nc.dram_tensor("w_k01", [3, 128, 128], BF16, kind="ExternalInput")
    wq2_d = nc.dram_tensor("w_q2", [3, 128, 64], BF16, kind="ExternalInput")
    wk2_d = nc.dram_tensor("w_k2", [3, 128, 64], BF16, kind="ExternalInput")
    wv_d = nc.dram_tensor("w_v", [3, 128, 192], BF16, kind="ExternalInput")
    wo_d = nc.dram_tensor("w_o", [3, 64, 384], BF16, kind="ExternalInput")
    um_d = nc.dram_tensor("umask", [4, 128, QT], BF16, kind="ExternalInput")
    id_d = nc.dram_tensor("ident", [128, 128], BF16, kind="ExternalInput")
    y_d = nc.dram_tensor("y", [t, C], F32, kind="ExternalOutput")
    # scratch for transposing the softmax denominator row into columns
    l_d = nc.dram_tensor("lscratch", [t // QT, 3, QT], F32)

    with tile.TileContext(nc) as tc, ExitStack() as ctx:
        persist = ctx.enter_context(tc.tile_pool(name="persist", bufs=1))

        # weights / masks / identity  (one-time loads on the scalar queue,
        # before the exp stream begins)
        wq01 = persist.tile([128, 3, 128], BF16)
        wk01 = persist.tile([128, 3, 128], BF16)
        wq2 = persist.tile([128, 3, 64], BF16)
        wk2 = persist.tile([128, 3, 64], BF16)
        wv = persist.tile([128, 3, 192], BF16)
        wo = persist.tile([64, 3, 384], BF16)
        for c in range(3):
            nc.scalar.dma_start(out=wq01[:, c, :], in_=wq01_d[c])
            nc.scalar.dma_start(out=wk01[:, c, :], in_=wk01_d[c])
            nc.scalar.dma_start(out=wq2[:, c, :], in_=wq2_d[c])
            nc.scalar.dma_start(out=wk2[:, c, :], in_=wk2_d[c])
            nc.scalar.dma_start(out=wv[:, c, :], in_=wv_d[c])
            nc.scalar.dma_start(out=wo[:, c, :], in_=wo_d[c])
        um = persist.tile([128, 4, QT], BF16)
        for j in range(4):
            nc.scalar.dma_start(out=um[:, j, :], in_=um_d[j])
        isb = persist.tile([128, 128], BF16)
        nc.scalar.dma_start(out=isb[:], in_=id_d[:, :])

        # persistent activations (bf16)
        qT01 = persist.tile([128, t], BF16)   # rows 0:64 h0 qT, 64:128 h1 qT
        kT01 = persist.tile([128, t], BF16)
        # head 2 q/k duplicated into both partition halves so chunk pairs
        # can run as concurrent row-group-packed matmuls
        qT2 = persist.tile([128, t], BF16)
        kT2 = persist.tile([128, t], BF16)
        vsb = persist.tile([128, nkc, 3, 65], BF16)  # [keys, chunk, head, d|one]
        nc.vector.memset(vsb[:, :, :, 64:65], 1.0)

        with (
            tc.tile_pool(name="xt", bufs=2) as xt_p,
            tc.tile_pool(name="exp_ps", bufs=2, space="PSUM") as exp_ps,
            tc.tile_pool(name="att_ps", bufs=3, space="PSUM") as att_ps,
            tc.tile_pool(name="sm_ps", bufs=1, space="PSUM") as sm_ps,
            tc.tile_pool(name="pth", bufs=6) as pth_p,
            tc.tile_pool(name="attn", bufs=6) as attn_p,
            tc.tile_pool(name="lrow", bufs=3) as lrow_p,
            tc.tile_pool(name="lcol", bufs=12) as lcol_p,
            tc.tile_pool(name="yout", bufs=3) as yout_p,
        ):
            # ---------------- "smalls" emission units ----------------
            def mk_xt(tb):
                xT = xt_p.tile([128, 3, QT], BF16, tag="xt", name="xT")
                for c in range(3):
                    nc.sync.dma_start(
                        out=xT[:, c, :],
                        in_=x_d[c * 128:(c + 1) * 128, tb * QT:(tb + 1) * QT],
                    )
                return xT

            def proj_qk_unit(tb, xT, wi):
                w_sb, m, dst = (
                    (wq01, 128, qT01),
                    (wk01, 128, kT01),
                    (wq2, 64, qT2),
                    (wk2, 64, kT2),
                )[wi]

                def emit():
                    ps = sm_ps.tile([128, QT], F32, tag="sm", name="psqk")
                    for c in range(3):
                        nc.tensor.matmul(
                            ps[0:m, :], w_sb[:, c, 0:m], xT[:, c, :],
                            start=(c == 0), stop=(c == 2),
                        )
                    nc.vector.tensor_copy(
                        dst[0:m, tb * QT:(tb + 1) * QT], ps[0:m, :]
                    )
                    if wi >= 2:
                        # replicate head-2 q/k into the other partition half
                        nc.gpsimd.dma_start(
                            out=dst[64:128, tb * QT:(tb + 1) * QT],
                            in_=dst[0:64, tb * QT:(tb + 1) * QT],
                        )
                return emit

            def proj_v_unit(tb, xT, s):
                def emit():
                    psv = sm_ps.tile([128, 3, 64], F32, tag="sm", name="psv")
                    for c in range(3):
                        nc.tensor.matmul(
                            psv[:, :, :].rearrange("p h d -> p (h d)"),
                            xT[:, c, s * 128:(s + 1) * 128],
                            wv[:, c, :],
                            start=(c == 0), stop=(c == 2),
                        )
                    nc.vector.tensor_copy(
                        vsb[:, tb * 4 + s, :, 0:64], psv[:, :, :]
                    )
                return emit

            def cproj_units(pqt, p_attn, p_linv):
                units = []
                for s in range(4):
                    def emit(s=s):
                        ysb = yout_p.tile([128, C], F32, tag="ysb", name="ysb")
                        for h in range(3):
                            yp = sm_ps.tile([128, C], F32, tag="sm", name="yp")
                            nc.tensor.matmul(
                                yp[:],
                                p_attn[h][:, s * 128:(s + 1) * 128],
                                wo[:, h, :],
                                start=True, stop=True,
                            )
                            sc = p_linv[h][:, s:s + 1]
                            if h == 0:
                                nc.vector.tensor_scalar(
                                    out=ysb[:], in0=yp[:], scalar1=sc,
                                    scalar2=None, op0=MULT,
                                )
                            else:
                                nc.vector.scalar_tensor_tensor(
                                    out=ysb[:], in0=yp[:], scalar=sc, in1=ysb[:],
                                    op0=MULT, op1=ADD,
                                )
                        nc.gpsimd.dma_start(
                            out=y_d[pqt * QT + s * 128:pqt * QT + (s + 1) * 128, :],
                            in_=ysb[:],
                        )
                    units.append(emit)
                return units

            # ---------------- phase B helpers ----------------
            def st_h01(qt, ck, q0, q1):
                """S^T + exp for h0/h1 of chunk ck -> pth tile."""
                dj = ck - 4 * qt          # >=0 on diagonal chunks
                trim = 128 * dj if dj >= 0 else 0
                ssx = exp_ps.tile([128, 2, QT], F32, tag="exp", name="ssx")
                for h in range(2):
                    nc.tensor.matmul(
                        ssx[:, h, trim:],
                        kT01[64 * h:64 * h + 64, ck * KC:(ck + 1) * KC],
                        qT01[64 * h:64 * h + 64, q0 + trim:q1],
                        start=True, stop=(dj < 0),
                    )
                if dj >= 0:
                    for h in range(2):
                        nc.tensor.matmul(
                            ssx[:, h, trim:], isb[:, :], um[:, dj, trim:],
                            start=False, stop=True,
                        )
                pth = pth_p.tile([128, 2, QT], BF16, tag="pth", name="pth")
                nc.scalar.activation(
                    out=pth[:, :, trim:], in_=ssx[:, :, trim:],
                    func=EXP, scale=SCALE,
                )
                return pth, trim

            def st_h2(qt, ck0, q0, q1, sch):
                """S^T + exp for h2 of chunks (ck0, ck0+1) -> pth tile."""
                ck1 = ck0 + 1
                ssc = exp_ps.tile([128, 2, QT], F32, tag="exp", name="ssc")
                for i, ck in ((0, ck0), (1, ck1)):
                    dj = ck - 4 * qt
                    nc.tensor.matmul(
                        ssc[:, i, :],
                        kT2[64 * i:64 * i + 64, ck * KC:(ck + 1) * KC],
                        qT2[64 * i:64 * i + 64, q0:q1],
                        start=True, stop=(dj < 0),
                    )
                    if dj >= 0:
                        nc.tensor.matmul(
                            ssc[:, i, :], isb[:, :], um[:, dj, :],
                            start=False, stop=True,
                        )
                pth = pth_p.tile([128, 2, QT], BF16, tag="pth", name="pth2")
                if sch:
                    nc.vector.tensor_scalar(
                        out=pth[:, :, :].bitcast(I16),
                        in0=ssc[:, :, :],
                        scalar1=SCH_K1, scalar2=SCH_K2,
                        op0=MULT, op1=ADD,
                    )
                else:
                    nc.scalar.activation(
                        out=pth[:, :, :], in_=ssc[:, :, :],
                        func=EXP, scale=SCALE,
                    )
                return pth

            def av_chunk(qt, ck, att3, pth01, trim01, pth2, i2, nch):
                for h in range(3):
                    if h < 2:
                        rhs = pth01[:, h, trim01:]
                        outp = att3[h][:, trim01:]
                    else:
                        rhs = pth2[:, i2, :]
                        outp = att3[2][:, :]
                    nc.tensor.matmul(
                        outp, vsb[:, ck, h, :], rhs,
                        start=(ck == 0), stop=(ck == nch - 1),
                    )

            # ---------------- main pipeline ----------------
            xT_cur = mk_xt(0)
            smalls = [proj_qk_unit(0, xT_cur, wi) for wi in range(4)]
            smalls += [proj_v_unit(0, xT_cur, s) for s in range(4)]
            for u in smalls:  # prologue: project block 0 densely
                u()

            prev_at = None  # (at_tiles, linv_tiles) of previous qt
            sch_ctr = 0

            for qt in range(nqt):
                nch = 4 * (qt + 1)
                ng = nch // 2
                q0, q1 = qt * QT, (qt + 1) * QT

                smalls = []
                if prev_at is not None:
                    smalls += cproj_units(qt - 1, *prev_at)
                if qt + 1 < nqt:
                    xT_nxt = mk_xt(qt + 1)
                    smalls += [proj_qk_unit(qt + 1, xT_nxt, wi) for wi in range(4)]
                    smalls += [proj_v_unit(qt + 1, xT_nxt, s) for s in range(4)]
                popped = 0

                att3 = [
                    att_ps.tile([65, QT], F32, tag="att", name=f"att{h}")
                    for h in range(3)
                ]
                pend = None
                for g in range(ng):
                    ck0, ck1 = 2 * g, 2 * g + 1
                    pthA0, trimA0 = st_h01(qt, ck0, q0, q1)
                    pthA1, trimA1 = st_h01(qt, ck1, q0, q1)
                    sch = SCH_NUM > 0 and (sch_ctr * SCH_NUM) % SCH_DEN < SCH_NUM
                    sch_ctr += 1
                    pthB = st_h2(qt, ck0, q0, q1, sch)
                    if pend is not None:
                        av_chunk(qt, *pend[0])
                        av_chunk(qt, *pend[1])
                    pend = (
                        (ck0, att3, pthA0, trimA0, pthB, 0, nch),
                        (ck1, att3, pthA1, trimA1, pthB, 1, nch),
                    )
                    # drip-feed fill work (c_proj of qt-1, projections of qt+1)
                    want = len(smalls) * (g + 1) // ng
                    while popped < want:
                        smalls[popped]()
                        popped += 1
                av_chunk(qt, *pend[0])
                av_chunk(qt, *pend[1])
                while popped < len(smalls):
                    smalls[popped]()
                    popped += 1

                # per-head normalization prep: extract denominator row,
                # transpose via DRAM roundtrip, reciprocal
                at_tiles = []
                linv_tiles = []
                for h in range(3):
                    at = attn_p.tile([64, QT], BF16, tag="attn", name="at")
                    attn_tiles_src = att3[h]
                    nc.vector.tensor_copy(at[:], attn_tiles_src[0:64, :])
                    at_tiles.append(at)
                    lrow = lrow_p.tile([65, QT], F32, tag="lrow", name="lrow")
                    nc.vector.tensor_copy(lrow[64:65, :], attn_tiles_src[64:65, :])
                    nc.sync.dma_start(out=l_d[qt, h], in_=lrow[64:65, :])
                    lcol = lcol_p.tile([128, 4], F32, tag="lcol", name="lcol")
                    nc.sync.dma_start(
                        out=lcol[:],
                        in_=l_d[qt, h].rearrange("(s p) -> p s", p=128),
                    )
                    linv = lcol_p.tile([128, 4], F32, tag="linv", name="linv")
                    linv_tiles.append(linv)
                    nc.vector.reciprocal(linv[:], lcol[:])
                prev_at = (at_tiles, linv_tiles)

            # epilogue: c_proj of the last q tile
            for u in cproj_units(nqt - 1, *prev_at):
                u()

    _split_excess_waits(nc)
    nc.finalize()
    return nc


_NC_CACHE = {}


def _get_nc(t=T):
    if t not in _NC_CACHE:
        _NC_CACHE[t] = build(t)
    return _NC_CACHE[t]


def _prep_core_inputs(x_b, w_attn, w_proj, hg, bf16):
    """Host-side shard prep for one core: batch x_b, head group hg (0/1)."""
    h0 = 3 * hg
    q = w_attn[:, 0:C]
    k = w_attn[:, C:2 * C]
    v = w_attn[:, 2 * C:3 * C]
    qcols = lambda h: q[:, h * D:(h + 1) * D]
    kcols = lambda h: k[:, h * D:(h + 1) * D]
    w_q01 = np.concatenate([qcols(h0), qcols(h0 + 1)], axis=1)      # [384,128]
    w_k01 = np.concatenate([kcols(h0), kcols(h0 + 1)], axis=1)
    w_q2 = qcols(h0 + 2)                                            # [384,64]
    w_k2 = kcols(h0 + 2)
    w_v = v[:, h0 * D:(h0 + 3) * D]                                 # [384,192]
    w_o = w_proj[h0 * D:(h0 + 3) * D, :]                            # [192,384]
    return {
        "xT16": np.ascontiguousarray(x_b.T, dtype=bf16),
        "w_q01": np.ascontiguousarray(w_q01.reshape(3, 128, 128), dtype=bf16),
        "w_k01": np.ascontiguousarray(w_k01.reshape(3, 128, 128), dtype=bf16),
        "w_q2": np.ascontiguousarray(w_q2.reshape(3, 128, 64), dtype=bf16),
        "w_k2": np.ascontiguousarray(w_k2.reshape(3, 128, 64), dtype=bf16),
        "w_v": np.ascontiguousarray(w_v.reshape(3, 128, 192), dtype=bf16),
        "w_o": np.ascontiguousarray(w_o.reshape(3, 64, 384), dtype=bf16),
    }


def _make_masks(bf16):
    """umask[j][p][f] = NEGM where key row p masks query col f, else 0."""
    m = np.zeros((4, 128, QT), dtype=np.float32)
    f = np.arange(QT)[None, :]
    p = np.arange(128)[:, None]
    for j in range(4):
        m[j] = np.where(f - 128 * j >= p, 0.0, NEGM)
    return m.astype(bf16)


def _make_ident(bf16):
    return np.eye(128, dtype=np.float32).astype(bf16)


def kernel(x, w_attn, w_proj):
    import ml_dtypes
    bf16 = ml_dtypes.bfloat16

    x = np.asarray(x, dtype=np.float32)
    w_attn = np.asarray(w_attn, dtype=np.float32)
    w_proj = np.asarray(w_proj, dtype=np.float32)
    b, t, c = x.shape

    nc = _get_nc(t)
    umask = _make_masks(bf16)
    ident = _make_ident(bf16)
    in_maps = []
    for core in range(8):
        im = _prep_core_inputs(x[core // 2], w_attn, w_proj, core % 2, bf16)
        im["umask"] = umask
        im["ident"] = ident
        in_maps.append(im)

    res = run_bass_kernel_spmd(nc, in_maps, list(range(8)))
    out = np.empty((b, t, c), dtype=np.float32)
    for bb in range(b):
        out[bb] = res.results[2 * bb]["y"] + res.results[2 * bb + 1]["y"]
    return out


# revision 14
# speedup vs baseline: 1.1953x; 1.0832x over previous
"""Causal self-attention Trainium2 kernel (B=4, T=4096, C=384, H=6).

Sharding: 8 cores = 4 batches x 2 head-groups (3 heads each). Each core
computes y_partial = attn(x[b], heads hg) @ w_proj[rows of hg]; the host
sums the two partials per batch (the "all-reduce after c_proj" done on
host during unshard).

v3 pipeline: flat software pipeline over (q-tile, chunk-pair) groups;
per-chunk AV into 3 persistent PSUM accumulators; causal masking as
identity x (-240*mask) accumulate-matmuls in PSUM; diagonal extent
trimming; proj/c_proj drip-fed through spare PSUM banks; single-DMA
batched transfers (each dma_start costs ~600ns on its queue); l
denominator roundtrip batched per q-tile on the gpsimd queue; optional
DVE bit-trick exp (Schraudolph int16->bf16) to offload the Scalar
engine, which is the global bottleneck at ~1 elem/cycle/partition.
"""

import numpy as np
from contextlib import ExitStack

import concourse.bass as bass
import concourse.tile as tile
from concourse import mybir
from concourse.bass_utils import run_bass_kernel_spmd
from concourse.vector_clock import ScopedClock

F32 = mybir.dt.float32
BF16 = mybir.dt.bfloat16
I16 = mybir.dt.int16
EXP = mybir.ActivationFunctionType.Exp
MULT = mybir.AluOpType.mult
ADD = mybir.AluOpType.add

B, T, C, H, D = 4, 4096, 384, 6, 64
HPC = 3            # heads per core
QT = 512           # q tile
KC = 128           # key chunk
SCALE = 1.0 / 8.0  # 1/sqrt(64)
NEGM = -240.0      # pre-scale mask bias: exp((S-240)/8) ~ 0

# DVE bit-trick exp (Schraudolph, direct to bf16 bits):
#   bf16_bits(exp(x/8)) ~= int16(x * (128*log2(e)/8) + K2)
SCH_K1 = 128.0 * 1.4426950408889634 / 8.0
SCH_K2 = 16250.75  # calibrated: ~3.3% max rel err on exp values
# Offload SCH_NUM of every SCH_DEN exp slots to the DVE (0 = off)
SCH_NUM = 0
SCH_DEN = 4


# ---------------------------------------------------------------------------
# Workaround: neuronxcc CoreV3 rejects >2 sem waits on the Tile tail drain.
# Split the drain's waits into individual sync-engine wait instructions.
def _drain_and_barrier_split(self, tick_clock, wait_clock):
    nc = self.nc
    drain_inst = nc.sync.drain()
    wait_clock.add_sem_waits(
        drain_inst.ins, ScopedClock({None: tick_clock.global_clock})
    )
    si = drain_inst.ins.sync_info
    if si is not None and si.on_wait and len(si.on_wait) > 1:
        waits = list(si.on_wait)
        si.on_wait = []
        allocated = {h.name: h for h in self.sems.allocated().values()}
        for w in waits:
            h = allocated.get(w.ant_name)
            assert h is not None, f"no sem handle for drain wait {w.ant_name}"
            assert w.wait_mode == "sem-ge-imm", w.wait_mode
            nc.sync.wait_ge(h, w.wait_value)
    nc.all_engine_barrier()
    assert self.sems is not None
    popped = nc._tile_sem_poison_stack.pop()
    assert popped is self._sem_poison
    nc.clear_and_free_semaphores(list(self.sems.allocated().values()))
    nc.all_engine_barrier()


tile.TileContext._drain_and_barrier = _drain_and_barrier_split


MAX_WAITS = 1  # CoreV3 per-instruction sem-wait capacity (S3_LW holds only 1)


def _split_excess_waits(nc):
    """Hoist sem waits beyond MAX_WAITS onto same-engine NOPs inserted
    directly before the over-limit instruction (waits are order-free)."""
    for fn in nc.m.functions:
        for bb in fn.blocks:
            insts = list(bb.instructions)
            out = []
            changed = False
            for inst in insts:
                si = inst.sync_info
                if si is not None and si.on_wait and len(si.on_wait) > MAX_WAITS:
                    waits = list(si.on_wait)
                    excess, keep = waits[:-MAX_WAITS], waits[-MAX_WAITS:]
                    si.on_wait = keep
                    inst.sync_info = si
                    for i in range(0, len(excess), MAX_WAITS):
                        nop = mybir.InstNoOp(
                            name=f"{inst.name}-waitsplit-{i}", ins=[], outs=[]
                        )
                        nop.engine = inst.engine
                        nop.sync_info = mybir.SyncInfo(
                            on_wait=excess[i:i + MAX_WAITS], on_update=[]
                        )
                        nc.register_instruction(nop)
                        out.append(nop)
                    changed = True
                out.append(inst)
            if changed:
                bb.instructions = out
# ---------------------------------------------------------------------------

# packed weight layout, per c-block: [q01 128 | k01 128 | q2 64 | k2 64 | v 192]
WOFF_Q01, WOFF_K01, WOFF_Q2, WOFF_K2, WOFF_V = 0, 128, 256, 320, 384
WCOLS = 576


def build(t=T):
    nqt = t // QT          # q tiles
    nkc = t // KC          # key chunks

    nc = bass.Bass()
    x_d = nc.dram_tensor("xT16", [C, t], BF16, kind="ExternalInput")
    wpk_d = nc.dram_tensor("wpack", [128, 3, WCOLS], BF16, kind="ExternalInput")
    wo_d = nc.dram_tensor("w_o", [64, 3, 384], BF16, kind="ExternalInput")
    um_d = nc.dram_tensor("umask", [128, 4, QT], BF16, kind="ExternalInput")
    id_d = nc.dram_tensor("ident", [128, 128], BF16, kind="ExternalInput")
    y_d = nc.dram_tensor("y", [t, C], F32, kind="ExternalOutput")
    # scratch for transposing the softmax denominator rows into columns
    l_d = nc.dram_tensor("lscratch", [t // QT, 3, QT], F32)

    with tile.TileContext(nc) as tc, ExitStack() as ctx:
        persist = ctx.enter_context(tc.tile_pool(name="persist", bufs=1))

        wpk = persist.tile([128, 3, WCOLS], BF16)
        wo = persist.tile([64, 3, 384], BF16)
        um = persist.tile([128, 4, QT], BF16)
        isb = persist.tile([128, 128], BF16)
        nc.scalar.dma_start(out=wpk[:], in_=wpk_d[:, :, :])
        nc.scalar.dma_start(out=wo[:], in_=wo_d[:, :, :])
        nc.scalar.dma_start(out=um[:], in_=um_d[:, :, :])
        nc.scalar.dma_start(out=isb[:], in_=id_d[:, :])

        # persistent activations (bf16)
        qT01 = persist.tile([128, t], BF16)   # rows 0:64 h0 qT, 64:128 h1 qT
        kT01 = persist.tile([128, t], BF16)
        # head 2 q/k duplicated into both partition halves so chunk pairs
        # can run as concurrent row-group-packed matmuls
        qT2 = persist.tile([128, t], BF16)
        kT2 = persist.tile([128, t], BF16)
        vsb = persist.tile([128, nkc, 3, 65], BF16)  # [keys, chunk, head, d|one]
        nc.vector.memset(vsb[:, :, :, 64:65], 1.0)

        with (
            tc.tile_pool(name="xt", bufs=2) as xt_p,
            tc.tile_pool(name="exp_ps", bufs=2, space="PSUM") as exp_ps,
            tc.tile_pool(name="att_ps", bufs=3, space="PSUM") as att_ps,
            tc.tile_pool(name="sm_ps", bufs=1, space="PSUM") as sm_ps,
            tc.tile_pool(name="pth", bufs=6) as pth_p,
            tc.tile_pool(name="attn", bufs=6) as attn_p,
            tc.tile_pool(name="lrow", bufs=2) as lrow_p,
            tc.tile_pool(name="lcol", bufs=4) as lcol_p,
            tc.tile_pool(name="yout", bufs=3) as yout_p,
        ):
            TAGOF = {id(exp_ps): "exp", id(att_ps): "att", id(sm_ps): "sm"}

            # ---------------- emission units ----------------
            def mk_xt(tb):
                xT = xt_p.tile([128, 3, QT], BF16, tag="xt", name="xT")
                nc.sync.dma_start(
                    out=xT[:],
                    in_=x_d[:, tb * QT:(tb + 1) * QT].rearrange(
                        "(c p) q -> p c q", p=128
                    ),
                )
                return xT

            def proj_qk_unit(tb, xT, wi, pool):
                off, m, dst = (
                    (WOFF_Q01, 128, qT01),
                    (WOFF_K01, 128, kT01),
                    (WOFF_Q2, 64, qT2),
                    (WOFF_K2, 64, kT2),
                )[wi]

                def emit():
                    ps = pool.tile([128, QT], F32, tag=TAGOF[id(pool)], name="psqk")
                    for c in range(3):
                        nc.tensor.matmul(
                            ps[0:m, :], wpk[:, c, off:off + m], xT[:, c, :],
                            start=(c == 0), stop=(c == 2),
                        )
                    nc.vector.tensor_copy(
                        dst[0:m, tb * QT:(tb + 1) * QT], ps[0:m, :]
                    )
                    if wi >= 2:
                        # replicate head-2 q/k into the other partition half
                        nc.gpsimd.dma_start(
                            out=dst[64:128, tb * QT:(tb + 1) * QT],
                            in_=dst[0:64, tb * QT:(tb + 1) * QT],
                        )
                return emit

            def proj_v_unit(tb, xT, s, pool):
                def emit():
                    psv = pool.tile([128, 3, 64], F32, tag=TAGOF[id(pool)], name="psv")
                    for c in range(3):
                        nc.tensor.matmul(
                            psv[:, :, :].rearrange("p h d -> p (h d)"),
                            xT[:, c, s * 128:(s + 1) * 128],
                            wpk[:, c, WOFF_V:WOFF_V + 192],
                            start=(c == 0), stop=(c == 2),
                        )
                    nc.vector.tensor_copy(
                        vsb[:, tb * 4 + s, :, 0:64], psv[:, :, :]
                    )
                return emit

            def cproj_units(pqt, p_attn, linv_cell, pools):
                units = []
                for s in range(4):
                    def emit(s=s):
                        p_linv = linv_cell["linv"]
                        ysb = yout_p.tile([128, C], F32, tag="ysb", name="ysb")
                        for h in range(3):
                            yp = pools[h].tile(
                                [128, C], F32, tag=TAGOF[id(pools[h])], name="yp"
                            )
                            nc.tensor.matmul(
                                yp[:],
                                p_attn[h][:, s * 128:(s + 1) * 128],
                                wo[:, h, :],
                                start=True, stop=True,
                            )
                            sc = p_linv[:, h, s:s + 1]
                            if h == 0:
                                nc.vector.tensor_scalar(
                                    out=ysb[:], in0=yp[:], scalar1=sc,
                                    scalar2=None, op0=MULT,
                                )
                            else:
                                nc.vector.scalar_tensor_tensor(
                                    out=ysb[:], in0=yp[:], scalar=sc, in1=ysb[:],
                                    op0=MULT, op1=ADD,
                                )
                        nc.gpsimd.dma_start(
                            out=y_d[pqt * QT + s * 128:pqt * QT + (s + 1) * 128, :],
                            in_=ysb[:],
                        )
                    units.append(emit)
                return units

            # ---------------- phase B helpers ----------------
            sch_state = [0]

            def use_sch():
                if SCH_NUM <= 0:
                    return False
                k = sch_state[0] % SCH_DEN
                sch_state[0] += 1
                return k < SCH_NUM

            def emit_exp(pth_ap, ssx_ap, sch):
                if sch:
                    nc.vector.tensor_scalar(
                        out=pth_ap.bitcast(I16), in0=ssx_ap,
                        scalar1=SCH_K1, scalar2=SCH_K2,
                        op0=MULT, op1=ADD,
                    )
                else:
                    nc.scalar.activation(
                        out=pth_ap, in_=ssx_ap, func=EXP, scale=SCALE,
                    )

            def st_h01(qt, ck, q0, q1):
                """S^T + exp for h0/h1 of chunk ck -> pth tile."""
                dj = ck - 4 * qt          # >=0 on diagonal chunks
                trim = 128 * dj if dj >= 0 else 0
                ssx = exp_ps.tile([128, 2, QT], F32, tag="exp", name="ssx")
                for h in range(2):
                    nc.tensor.matmul(
                        ssx[:, h, trim:],
                        kT01[64 * h:64 * h + 64, ck * KC:(ck + 1) * KC],
                        qT01[64 * h:64 * h + 64, q0 + trim:q1],
                        start=True, stop=(dj < 0),
                    )
                if dj >= 0:
                    for h in range(2):
                        nc.tensor.matmul(
                            ssx[:, h, trim:], isb[:, :], um[:, dj, trim:],
                            start=False, stop=True,
                        )
                pth = pth_p.tile([128, 2, QT], BF16, tag="pth", name="pth")
                emit_exp(pth[:, :, trim:], ssx[:, :, trim:], use_sch())
                return pth, trim

            def st_h2(qt, ck0, q0, q1):
                """S^T + exp for h2 of chunks (ck0, ck0+1) -> pth tile."""
                ck1 = ck0 + 1
                ssc = exp_ps.tile([128, 2, QT], F32, tag="exp", name="ssc")
                for i, ck in ((0, ck0), (1, ck1)):
                    dj = ck - 4 * qt
                    nc.tensor.matmul(
                        ssc[:, i, :],
                        kT2[64 * i:64 * i + 64, ck * KC:(ck + 1) * KC],
                        qT2[64 * i:64 * i + 64, q0:q1],
                        start=True, stop=(dj < 0),
                    )
                    if dj >= 0:
                        nc.tensor.matmul(
                            ssc[:, i, :], isb[:, :], um[:, dj, :],
                            start=False, stop=True,
                        )
                pth = pth_p.tile([128, 2, QT], BF16, tag="pth", name="pth2")
                emit_exp(pth[:, :, :], ssc[:, :, :], use_sch())
                return pth

            def av_chunk(ck, att3, pth01, trim01, pth2, i2, nch):
                for h in range(3):
                    if h < 2:
                        rhs = pth01[:, h, trim01:]
                        outp = att3[h][:, trim01:]
                    else:
                        rhs = pth2[:, i2, :]
                        outp = att3[2][:, :]
                    nc.tensor.matmul(
                        outp, vsb[:, ck, h, :], rhs,
                        start=(ck == 0), stop=(ck == nch - 1),
                    )

            # ---------------- main pipeline ----------------
            from collections import deque
            smalls = deque()
            ps_rr = [exp_ps, att_ps, sm_ps, att_ps, exp_ps, att_ps, sm_ps, att_ps]

            xT_cur = mk_xt(0)
            units0 = [proj_qk_unit(0, xT_cur, wi, ps_rr[wi]) for wi in range(4)]
            units0 += [proj_v_unit(0, xT_cur, s, ps_rr[4 + s]) for s in range(4)]
            for u in units0:  # prologue: project block 0 across 6 PSUM banks
                u()

            pend = None      # deferred AV work: (spec0, spec1, qt, is_last)
            att_by_qt = {}

            def emit_denominators(qtp):
                """att rows -> at tiles + batched l roundtrip -> linv."""
                att3 = att_by_qt.pop(qtp)
                at_tiles = []
                lrow = lrow_p.tile([65, 3, QT], F32, tag="lrow", name="lrow")
                for h in range(3):
                    at = attn_p.tile([64, QT], BF16, tag="attn", name="at")
                    nc.vector.tensor_copy(at[:], att3[h][0:64, :])
                    at_tiles.append(at)
                    nc.vector.tensor_copy(lrow[64:65, h, :], att3[h][64:65, :])
                nc.gpsimd.dma_start(out=l_d[qtp], in_=lrow[64:65, :, :])
                lcol = lcol_p.tile([128, 3, 4], F32, tag="lcol", name="lcol")
                nc.gpsimd.dma_start(
                    out=lcol[:],
                    in_=l_d[qtp].rearrange("h (s p) -> p h s", p=128),
                )
                return at_tiles, lcol

            def flush_pend(tail=False):
                nonlocal pend
                if pend is None:
                    return
                spec0, spec1, qtp, last = pend
                pend = None
                av_chunk(*spec0)
                av_chunk(*spec1)
                if last:
                    at_tiles, lcol = emit_denominators(qtp)
                    linv_cell = {}

                    def recip_unit(lcol=lcol, cell=linv_cell):
                        # deferred so the DVE queue isn't stalled on the
                        # l DRAM roundtrip at the q-tile boundary
                        linv = lcol_p.tile([128, 3, 4], F32, tag="lcol",
                                           name="linv")
                        nc.vector.reciprocal(linv[:], lcol[:])
                        cell["linv"] = linv

                    pools = (
                        (att_ps, att_ps, att_ps) if tail
                        else (sm_ps, sm_ps, sm_ps)
                    )
                    smalls.append(recip_unit)
                    smalls.extend(
                        cproj_units(qtp, at_tiles, linv_cell, pools))

            for qt in range(nqt):
                nch = 4 * (qt + 1)
                ng = nch // 2
                q0, q1 = qt * QT, (qt + 1) * QT

                if qt + 1 < nqt:
                    xT_nxt = mk_xt(qt + 1)
                    due = [proj_qk_unit(qt + 1, xT_nxt, wi, sm_ps)
                           for wi in range(4)]
                    due += [proj_v_unit(qt + 1, xT_nxt, s, sm_ps)
                            for s in range(4)]
                    smalls.extend(due)

                att3 = [
                    att_ps.tile([65, QT], F32, tag="att", name=f"att{h}")
                    for h in range(3)
                ]
                att_by_qt[qt] = att3

                for g in range(ng):
                    ck0, ck1 = 2 * g, 2 * g + 1
                    pthA0, trimA0 = st_h01(qt, ck0, q0, q1)
                    pthA1, trimA1 = st_h01(qt, ck1, q0, q1)
                    pthB = st_h2(qt, ck0, q0, q1)
                    flush_pend()
                    pend = (
                        (ck0, att3, pthA0, trimA0, pthB, 0, nch),
                        (ck1, att3, pthA1, trimA1, pthB, 1, nch),
                        qt, g == ng - 1,
                    )
                    npop = 2 if len(smalls) < 2 * (ng - g) else 3
                    for _ in range(min(npop, len(smalls))):
                        smalls.popleft()()
                # proj units for qt+1 must land before qt+1's S^T reads them
                if qt + 1 < nqt:
                    while smalls:
                        smalls.popleft()()

            flush_pend(tail=True)   # AV of last group + denominators of qt 7
            while smalls:           # c_proj of the last q tile
                smalls.popleft()()

    _split_excess_waits(nc)
    nc.finalize()
    return nc


_NC_CACHE = {}


def _get_nc(t=T):
    if t not in _NC_CACHE:
        _NC_CACHE[t] = build(t)
    return _NC_CACHE[t]


def _prep_core_inputs(x_b, w_attn, w_proj, hg, bf16):
    """Host-side shard prep for one core: batch x_b, head group hg (0/1)."""
    h0 = 3 * hg
    q = w_attn[:, 0:C]
    k = w_attn[:, C:2 * C]
    v = w_attn[:, 2 * C:3 * C]
    qcols = lambda h: q[:, h * D:(h + 1) * D]
    kcols = lambda h: k[:, h * D:(h + 1) * D]
    w_q01 = np.concatenate([qcols(h0), qcols(h0 + 1)], axis=1)      # [384,128]
    w_k01 = np.concatenate([kcols(h0), kcols(h0 + 1)], axis=1)
    w_q2 = qcols(h0 + 2)                                            # [384,64]
    w_k2 = kcols(h0 + 2)
    w_v = v[:, h0 * D:(h0 + 3) * D]                                 # [384,192]
    w_o = w_proj[h0 * D:(h0 + 3) * D, :]                            # [192,384]
    # pack per c-block: [q01 | k01 | q2 | k2 | v]
    wpack = np.concatenate([w_q01, w_k01, w_q2, w_k2, w_v], axis=1)  # [384,576]
    wpack = wpack.reshape(3, 128, WCOLS).transpose(1, 0, 2)          # [128,3,576]
    wo3 = np.stack([w_o[h * D:(h + 1) * D] for h in range(3)], axis=1)
    return {
        "xT16": np.ascontiguousarray(x_b.T, dtype=bf16),
        "wpack": np.ascontiguousarray(wpack, dtype=bf16),
        "w_o": np.ascontiguousarray(wo3, dtype=bf16),               # [64,3,384]
    }


def _make_masks(bf16):
    """umask[p][j][f] = NEGM where key row p masks query col f, else 0."""
    m = np.zeros((128, 4, QT), dtype=np.float32)
    f = np.arange(QT)[None, :]
    p = np.arange(128)[:, None]
    for j in range(4):
        m[:, j, :] = np.where(f - 128 * j >= p, 0.0, NEGM)
    return m.astype(bf16)


def _make_ident(bf16):
    return np.eye(128, dtype=np.float32).astype(bf16)


def kernel(x, w_attn, w_proj):
    import ml_dtypes
    bf16 = ml_dtypes.bfloat16

    x = np.asarray(x, dtype=np.float32)
    w_attn = np.asarray(w_attn, dtype=np.float32)
    w_proj = np.asarray(w_proj, dtype=np.float32)
    b, t, c = x.shape

    nc = _get_nc(t)
    umask = _make_masks(bf16)
    ident = _make_ident(bf16)
    in_maps = []
    for core in range(8):
        im = _prep_core_inputs(x[core // 2], w_attn, w_proj, core % 2, bf16)
        im["umask"] = umask
        im["ident"] = ident
        in_maps.append(im)

    res = run_bass_kernel_spmd(nc, in_maps, list(range(8)))
    out = np.empty((b, t, c), dtype=np.float32)
    for bb in range(b):
        out[bb] = res.results[2 * bb]["y"] + res.results[2 * bb + 1]["y"]
    return out


# revision 23
# speedup vs baseline: 1.3265x; 1.1098x over previous
"""Causal self-attention Trainium2 kernel (B=4, T=4096, C=384, H=6).

Sharding: 8 cores = 4 batches x 2 head-groups (3 heads each). Each core
computes y_partial = attn(x[b], heads hg) @ w_proj[rows of hg]; the host
sums the two partials per batch (the "all-reduce after c_proj" done on
host during unshard).

v3 pipeline: flat software pipeline over (q-tile, chunk-pair) groups;
per-chunk AV into 3 persistent PSUM accumulators; causal masking as
identity x (-240*mask) accumulate-matmuls in PSUM; diagonal extent
trimming; proj/c_proj drip-fed through spare PSUM banks; single-DMA
batched transfers (each dma_start costs ~600ns on its queue); l
denominator roundtrip batched per q-tile on the gpsimd queue; optional
DVE bit-trick exp (Schraudolph int16->bf16) to offload the Scalar
engine, which is the global bottleneck at ~1 elem/cycle/partition.
"""

import numpy as np
from contextlib import ExitStack

import concourse.bass as bass
import concourse.tile as tile
from concourse import mybir
from concourse.bass_utils import run_bass_kernel_spmd
from concourse.vector_clock import ScopedClock

F32 = mybir.dt.float32
BF16 = mybir.dt.bfloat16
I16 = mybir.dt.int16
EXP = mybir.ActivationFunctionType.Exp
MULT = mybir.AluOpType.mult
ADD = mybir.AluOpType.add

B, T, C, H, D = 4, 4096, 384, 6, 64
HPC = 3            # heads per core
QT = 512           # q tile
KC = 128           # key chunk
SCALE = 1.0 / 8.0  # 1/sqrt(64)
NEGM = -240.0      # pre-scale mask bias: exp((S-240)/8) ~ 0

# DVE bit-trick exp (Schraudolph, direct to bf16 bits):
#   bf16_bits(exp(x/8)) ~= int16(x * (128*log2(e)/8) + K2)
SCH_K1 = 128.0 * 1.4426950408889634 / 8.0
SCH_K2 = 16250.75  # calibrated: ~3.3% max rel err on exp values
# Offload SCH_NUM of every SCH_DEN exp slots to the DVE (0 = off)
SCH_NUM = 0
SCH_DEN = 4


# ---------------------------------------------------------------------------
# Workaround: neuronxcc CoreV3 rejects >2 sem waits on the Tile tail drain.
# Split the drain's waits into individual sync-engine wait instructions.
def _drain_and_barrier_split(self, tick_clock, wait_clock):
    nc = self.nc
    drain_inst = nc.sync.drain()
    wait_clock.add_sem_waits(
        drain_inst.ins, ScopedClock({None: tick_clock.global_clock})
    )
    si = drain_inst.ins.sync_info
    if si is not None and si.on_wait and len(si.on_wait) > 1:
        waits = list(si.on_wait)
        si.on_wait = []
        allocated = {h.name: h for h in self.sems.allocated().values()}
        for w in waits:
            h = allocated.get(w.ant_name)
            assert h is not None, f"no sem handle for drain wait {w.ant_name}"
            assert w.wait_mode == "sem-ge-imm", w.wait_mode
            nc.sync.wait_ge(h, w.wait_value)
    nc.all_engine_barrier()
    assert self.sems is not None
    popped = nc._tile_sem_poison_stack.pop()
    assert popped is self._sem_poison
    nc.clear_and_free_semaphores(list(self.sems.allocated().values()))
    nc.all_engine_barrier()


tile.TileContext._drain_and_barrier = _drain_and_barrier_split


MAX_WAITS = 1  # CoreV3 per-instruction sem-wait capacity (S3_LW holds only 1)


def _split_excess_waits(nc):
    """Hoist sem waits beyond MAX_WAITS onto same-engine NOPs inserted
    directly before the over-limit instruction (waits are order-free)."""
    for fn in nc.m.functions:
        for bb in fn.blocks:
            insts = list(bb.instructions)
            out = []
            changed = False
            for inst in insts:
                si = inst.sync_info
                if si is not None and si.on_wait and len(si.on_wait) > MAX_WAITS:
                    waits = list(si.on_wait)
                    excess, keep = waits[:-MAX_WAITS], waits[-MAX_WAITS:]
                    si.on_wait = keep
                    inst.sync_info = si
                    for i in range(0, len(excess), MAX_WAITS):
                        nop = mybir.InstNoOp(
                            name=f"{inst.name}-waitsplit-{i}", ins=[], outs=[]
                        )
                        nop.engine = inst.engine
                        nop.sync_info = mybir.SyncInfo(
                            on_wait=excess[i:i + MAX_WAITS], on_update=[]
                        )
                        nc.register_instruction(nop)
                        out.append(nop)
                    changed = True
                out.append(inst)
            if changed:
                bb.instructions = out
# ---------------------------------------------------------------------------

# packed weight layout, per c-block: [q01 128 | k01 128 | q2 64 | k2 64 | v 192]
WOFF_Q01, WOFF_K01, WOFF_Q2, WOFF_K2, WOFF_V = 0, 128, 256, 320, 384
WCOLS = 576


def build(t=T):
    nqt = t // QT          # q tiles
    nkc = t // KC          # key chunks

    nc = bass.Bass()
    x_d = nc.dram_tensor("xT16", [C, t], BF16, kind="ExternalInput")
    wpk_d = nc.dram_tensor("wpack", [128, 3, WCOLS], BF16, kind="ExternalInput")
    wo_d = nc.dram_tensor("w_o", [64, 3, 384], BF16, kind="ExternalInput")
    # narrow 0/1 causal mask for the 128-wide diagonal boundary block
    # (j-independent: allow iff in-block col >= key row), replicated x2 so
    # one tensor_tensor covers both h01 sub-tiles
    um_d = nc.dram_tensor("umask", [128, 2, KC], BF16, kind="ExternalInput")
    y_d = nc.dram_tensor("y", [t, C], F32, kind="ExternalOutput")
    # scratch for transposing the softmax denominator rows into columns
    l_d = nc.dram_tensor("lscratch", [t // QT, 3, QT], F32)

    with tile.TileContext(nc) as tc, ExitStack() as ctx:
        persist = ctx.enter_context(tc.tile_pool(name="persist", bufs=1))

        wpk = persist.tile([128, 3, WCOLS], BF16)
        wo = persist.tile([64, 3, 384], BF16)
        um = persist.tile([128, 2, KC], BF16)
        nc.scalar.dma_start(out=wpk[:], in_=wpk_d[:, :, :])
        nc.scalar.dma_start(out=wo[:], in_=wo_d[:, :, :])
        nc.scalar.dma_start(out=um[:], in_=um_d[:, :, :])

        # persistent activations (bf16)
        qT01 = persist.tile([128, t], BF16)   # rows 0:64 h0 qT, 64:128 h1 qT
        kT01 = persist.tile([128, t], BF16)
        # head 2 q/k duplicated into both partition halves so chunk pairs
        # can run as concurrent row-group-packed matmuls
        qT2 = persist.tile([128, t], BF16)
        kT2 = persist.tile([128, t], BF16)
        vsb = persist.tile([128, nkc, 3, 65], BF16)  # [keys, chunk, head, d|one]
        nc.vector.memset(vsb[:, :, :, 64:65], 1.0)

        with (
            tc.tile_pool(name="xt", bufs=2) as xt_p,
            tc.tile_pool(name="exp_ps", bufs=2, space="PSUM") as exp_ps,
            tc.tile_pool(name="att_ps", bufs=3, space="PSUM") as att_ps,
            tc.tile_pool(name="sm_ps", bufs=1, space="PSUM") as sm_ps,
            tc.tile_pool(name="pth", bufs=6) as pth_p,
            tc.tile_pool(name="attn", bufs=6) as attn_p,
            tc.tile_pool(name="lrow", bufs=2) as lrow_p,
            tc.tile_pool(name="lcol", bufs=4) as lcol_p,
            tc.tile_pool(name="yout", bufs=3) as yout_p,
        ):
            TAGOF = {id(exp_ps): "exp", id(att_ps): "att", id(sm_ps): "sm"}

            # ---------------- emission units ----------------
            def mk_xt(tb):
                xT = xt_p.tile([128, 3, QT], BF16, tag="xt", name="xT")
                nc.sync.dma_start(
                    out=xT[:],
                    in_=x_d[:, tb * QT:(tb + 1) * QT].rearrange(
                        "(c p) q -> p c q", p=128
                    ),
                )
                return xT

            def proj_qk_unit(tb, xT, wi, pool):
                # wi 0: q01, 1: k01, 2: q2|k2 merged in one 128-col stationary
                off, dst01 = (
                    (WOFF_Q01, qT01),
                    (WOFF_K01, kT01),
                    (WOFF_Q2, None),
                )[wi]

                def emit():
                    ps = pool.tile([128, QT], F32, tag=TAGOF[id(pool)], name="psqk")
                    for c in range(3):
                        nc.tensor.matmul(
                            ps[:, :], wpk[:, c, off:off + 128], xT[:, c, :],
                            start=(c == 0), stop=(c == 2),
                        )
                    blk = slice(tb * QT, (tb + 1) * QT)
                    if dst01 is not None:
                        nc.vector.tensor_copy(dst01[:, blk], ps[:, :])
                    else:
                        # rows 0:64 = head-2 q, rows 64:128 = head-2 k;
                        # replicate each into both partition halves
                        nc.vector.tensor_copy(qT2[0:64, blk], ps[0:64, :])
                        nc.vector.tensor_copy(kT2[0:64, blk], ps[64:128, :])
                        nc.gpsimd.dma_start(
                            out=qT2[64:128, blk], in_=qT2[0:64, blk])
                        nc.gpsimd.dma_start(
                            out=kT2[64:128, blk], in_=kT2[0:64, blk])
                return emit

            def proj_v_unit(tb, xT, s, pool):
                def emit():
                    psv = pool.tile([128, 3, 64], F32, tag=TAGOF[id(pool)], name="psv")
                    for c in range(3):
                        nc.tensor.matmul(
                            psv[:, :, :].rearrange("p h d -> p (h d)"),
                            xT[:, c, s * 128:(s + 1) * 128],
                            wpk[:, c, WOFF_V:WOFF_V + 192],
                            start=(c == 0), stop=(c == 2),
                        )
                    nc.vector.tensor_copy(
                        vsb[:, tb * 4 + s, :, 0:64], psv[:, :, :]
                    )
                return emit

            def cproj_units(pqt, p_attn, linv_cell, pools):
                units = []
                for s in range(4):
                    def emit(s=s):
                        p_linv = linv_cell["linv"]
                        ysb = yout_p.tile([128, C], F32, tag="ysb", name="ysb")
                        for h in range(3):
                            yp = pools[h].tile(
                                [128, C], F32, tag=TAGOF[id(pools[h])], name="yp"
                            )
                            nc.tensor.matmul(
                                yp[:],
                                p_attn[h][:, s * 128:(s + 1) * 128],
                                wo[:, h, :],
                                start=True, stop=True,
                            )
                            sc = p_linv[:, h, s:s + 1]
                            if h == 0:
                                nc.vector.tensor_scalar(
                                    out=ysb[:], in0=yp[:], scalar1=sc,
                                    scalar2=None, op0=MULT,
                                )
                            else:
                                nc.vector.scalar_tensor_tensor(
                                    out=ysb[:], in0=yp[:], scalar=sc, in1=ysb[:],
                                    op0=MULT, op1=ADD,
                                )
                        nc.gpsimd.dma_start(
                            out=y_d[pqt * QT + s * 128:pqt * QT + (s + 1) * 128, :],
                            in_=ysb[:],
                        )
                    units.append(emit)
                return units

            # ---------------- phase B helpers ----------------
            sch_state = [0]

            def use_sch():
                if SCH_NUM <= 0:
                    return False
                k = sch_state[0] % SCH_DEN
                sch_state[0] += 1
                return k < SCH_NUM

            def emit_exp(pth_ap, ssx_ap, sch):
                if sch:
                    nc.vector.tensor_scalar(
                        out=pth_ap.bitcast(I16), in0=ssx_ap,
                        scalar1=SCH_K1, scalar2=SCH_K2,
                        op0=MULT, op1=ADD,
                    )
                else:
                    nc.scalar.activation(
                        out=pth_ap, in_=ssx_ap, func=EXP, scale=SCALE,
                    )

            def st_h01(qt, ck, q0, q1):
                """S^T + exp (+ diagonal mask) for h0/h1 of chunk ck."""
                dj = ck - 4 * qt          # >=0 on diagonal chunks
                trim = 128 * dj if dj >= 0 else 0
                ssx = exp_ps.tile([128, 2, QT], F32, tag="exp", name="ssx")
                for h in range(2):
                    nc.tensor.matmul(
                        ssx[:, h, trim:],
                        kT01[64 * h:64 * h + 64, ck * KC:(ck + 1) * KC],
                        qT01[64 * h:64 * h + 64, q0 + trim:q1],
                        start=True, stop=True,
                    )
                pth = pth_p.tile([128, 2, QT], BF16, tag="pth", name="pth")
                emit_exp(pth[:, :, trim:], ssx[:, :, trim:], use_sch())
                if dj >= 0:
                    # zero the upper-triangular part of the boundary block
                    sl = pth[:, :, trim:trim + KC]
                    nc.vector.tensor_tensor(
                        out=sl, in0=sl, in1=um[:, :, :], op=MULT,
                    )
                return pth, trim

            def st_h2(qt, ck0, q0, q1):
                """S^T + exp (+ diagonal mask) for h2 of chunks (ck0, ck0+1)."""
                ssc = exp_ps.tile([128, 2, QT], F32, tag="exp", name="ssc")
                trims = []
                for i, ck in ((0, ck0), (1, ck0 + 1)):
                    dj = ck - 4 * qt
                    tr = 128 * dj if dj >= 0 else 0
                    trims.append(tr)
                    nc.tensor.matmul(
                        ssc[:, i, tr:],
                        kT2[64 * i:64 * i + 64, ck * KC:(ck + 1) * KC],
                        qT2[64 * i:64 * i + 64, q0 + tr:q1],
                        start=True, stop=True,
                    )
                pth = pth_p.tile([128, 2, QT], BF16, tag="pth", name="pth2")
                emit_exp(pth[:, :, trims[0]:], ssc[:, :, trims[0]:], use_sch())
                for i, ck in ((0, ck0), (1, ck0 + 1)):
                    dj = ck - 4 * qt
                    if dj >= 0:
                        tr = 128 * dj
                        sl = pth[:, i, tr:tr + KC]
                        nc.vector.tensor_tensor(
                            out=sl, in0=sl, in1=um[:, 0, :], op=MULT,
                        )
                return pth, trims

            def av_chunk(ck, att3, pth01, trim01, pth2, i2, trim2, nch):
                for h in range(3):
                    if h < 2:
                        rhs = pth01[:, h, trim01:]
                        outp = att3[h][:, trim01:]
                    else:
                        rhs = pth2[:, i2, trim2:]
                        outp = att3[2][:, trim2:]
                    nc.tensor.matmul(
                        outp, vsb[:, ck, h, :], rhs,
                        start=(ck == 0), stop=(ck == nch - 1),
                    )

            # ---------------- main pipeline ----------------
            from collections import deque
            smalls = deque()
            ps_rr = [exp_ps, att_ps, sm_ps, att_ps, exp_ps, att_ps, sm_ps, att_ps]

            xT_cur = mk_xt(0)
            units0 = [proj_qk_unit(0, xT_cur, wi, ps_rr[wi]) for wi in range(3)]
            units0 += [proj_v_unit(0, xT_cur, s, ps_rr[3 + s]) for s in range(4)]
            for u in units0:  # prologue: project block 0 across 6 PSUM banks
                u()

            pend = None      # deferred AV work: (spec0, spec1, qt, is_last)
            att_by_qt = {}

            def emit_denominators(qtp):
                """att rows -> at tiles + batched l roundtrip -> linv."""
                att3 = att_by_qt.pop(qtp)
                at_tiles = []
                lrow = lrow_p.tile([65, 3, QT], F32, tag="lrow", name="lrow")
                for h in range(3):
                    at = attn_p.tile([64, QT], BF16, tag="attn", name="at")
                    nc.vector.tensor_copy(at[:], att3[h][0:64, :])
                    at_tiles.append(at)
                    nc.vector.tensor_copy(lrow[64:65, h, :], att3[h][64:65, :])
                nc.gpsimd.dma_start(out=l_d[qtp], in_=lrow[64:65, :, :])
                lcol = lcol_p.tile([128, 3, 4], F32, tag="lcol", name="lcol")
                nc.gpsimd.dma_start(
                    out=lcol[:],
                    in_=l_d[qtp].rearrange("h (s p) -> p h s", p=128),
                )
                return at_tiles, lcol

            def flush_pend(tail=False):
                nonlocal pend
                if pend is None:
                    return
                spec0, spec1, qtp, last = pend
                pend = None
                av_chunk(*spec0)
                av_chunk(*spec1)
                if last:
                    at_tiles, lcol = emit_denominators(qtp)
                    linv_cell = {}

                    def recip_unit(lcol=lcol, cell=linv_cell):
                        # deferred so the DVE queue isn't stalled on the
                        # l DRAM roundtrip at the q-tile boundary
                        linv = lcol_p.tile([128, 3, 4], F32, tag="lcol",
                                           name="linv")
                        nc.vector.reciprocal(linv[:], lcol[:])
                        cell["linv"] = linv

                    pools = (
                        (att_ps, att_ps, att_ps) if tail
                        else (sm_ps, sm_ps, sm_ps)
                    )
                    smalls.append(recip_unit)
                    smalls.extend(
                        cproj_units(qtp, at_tiles, linv_cell, pools))

            for qt in range(nqt):
                nch = 4 * (qt + 1)
                ng = nch // 2
                q0, q1 = qt * QT, (qt + 1) * QT

                if qt + 1 < nqt:
                    xT_nxt = mk_xt(qt + 1)
                    due = [proj_qk_unit(qt + 1, xT_nxt, wi, sm_ps)
                           for wi in range(3)]
                    due += [proj_v_unit(qt + 1, xT_nxt, s, sm_ps)
                            for s in range(4)]
                    smalls.extend(due)

                att3 = [
                    att_ps.tile([65, QT], F32, tag="att", name=f"att{h}")
                    for h in range(3)
                ]
                att_by_qt[qt] = att3

                for g in range(ng):
                    ck0, ck1 = 2 * g, 2 * g + 1
                    pthA0, trimA0 = st_h01(qt, ck0, q0, q1)
                    pthA1, trimA1 = st_h01(qt, ck1, q0, q1)
                    pthB, trimsB = st_h2(qt, ck0, q0, q1)
                    flush_pend()
                    pend = (
                        (ck0, att3, pthA0, trimA0, pthB, 0, trimsB[0], nch),
                        (ck1, att3, pthA1, trimA1, pthB, 1, trimsB[1], nch),
                        qt, g == ng - 1,
                    )
                    npop = 2 if len(smalls) < 2 * (ng - g) else 3
                    for _ in range(min(npop, len(smalls))):
                        smalls.popleft()()
                # proj units for qt+1 must land before qt+1's S^T reads them
                if qt + 1 < nqt:
                    while smalls:
                        smalls.popleft()()

            flush_pend(tail=True)   # AV of last group + denominators of qt 7
            while smalls:           # c_proj of the last q tile
                smalls.popleft()()

    _split_excess_waits(nc)
    nc.finalize()
    return nc


_NC_CACHE = {}


def _get_nc(t=T):
    if t not in _NC_CACHE:
        _NC_CACHE[t] = build(t)
    return _NC_CACHE[t]


def _prep_core_inputs(x_b, w_attn, w_proj, hg, bf16):
    """Host-side shard prep for one core: batch x_b, head group hg (0/1)."""
    h0 = 3 * hg
    q = w_attn[:, 0:C]
    k = w_attn[:, C:2 * C]
    v = w_attn[:, 2 * C:3 * C]
    qcols = lambda h: q[:, h * D:(h + 1) * D]
    kcols = lambda h: k[:, h * D:(h + 1) * D]
    w_q01 = np.concatenate([qcols(h0), qcols(h0 + 1)], axis=1)      # [384,128]
    w_k01 = np.concatenate([kcols(h0), kcols(h0 + 1)], axis=1)
    w_q2 = qcols(h0 + 2)                                            # [384,64]
    w_k2 = kcols(h0 + 2)
    w_v = v[:, h0 * D:(h0 + 3) * D]                                 # [384,192]
    w_o = w_proj[h0 * D:(h0 + 3) * D, :]                            # [192,384]
    # pack per c-block: [q01 | k01 | q2 | k2 | v]
    wpack = np.concatenate([w_q01, w_k01, w_q2, w_k2, w_v], axis=1)  # [384,576]
    wpack = wpack.reshape(3, 128, WCOLS).transpose(1, 0, 2)          # [128,3,576]
    wo3 = np.stack([w_o[h * D:(h + 1) * D] for h in range(3)], axis=1)
    return {
        "xT16": np.ascontiguousarray(x_b.T, dtype=bf16),
        "wpack": np.ascontiguousarray(wpack, dtype=bf16),
        "w_o": np.ascontiguousarray(wo3, dtype=bf16),               # [64,3,384]
    }


def _make_masks(bf16):
    """umask[p][r][f] = 1.0 where boundary-block col f >= key row p."""
    f = np.arange(KC)[None, :]
    p = np.arange(128)[:, None]
    m = (f >= p).astype(np.float32)          # [128, KC]
    return np.ascontiguousarray(
        np.repeat(m[:, None, :], 2, axis=1)).astype(bf16)


def kernel(x, w_attn, w_proj):
    import ml_dtypes
    bf16 = ml_dtypes.bfloat16

    x = np.asarray(x, dtype=np.float32)
    w_attn = np.asarray(w_attn, dtype=np.float32)
    w_proj = np.asarray(w_proj, dtype=np.float32)
    b, t, c = x.shape

    nc = _get_nc(t)
    umask = _make_masks(bf16)
    in_maps = []
    for core in range(8):
        im = _prep_core_inputs(x[core // 2], w_attn, w_proj, core % 2, bf16)
        im["umask"] = umask
        in_maps.append(im)

    res = run_bass_kernel_spmd(nc, in_maps, list(range(8)))
    out = np.empty((b, t, c), dtype=np.float32)
    for bb in range(b):
        out[bb] = res.results[2 * bb]["y"] + res.results[2 * bb + 1]["y"]
    return out
